# revision 8
# baseline (speedup 1.0000x reference)
"""Trainium2 Bass kernel for nn_BilinearFeedForward.

Math (per batch element b, reference semantics):
    q_r = x @ Wqr ; q_i = x @ Wqi ; query = relu(q_r) * relu(q_i)
    k = x @ Wk ; v = x @ Wv
    k /= (||k||_2 over n + eps) ; v /= (||v||_2 over n + eps)   (per column)
    kv = relu(k^T v)            [D, D]
    out = query @ kv            [N, D]

Key algebraic restructuring: with G = x^T x (symmetric, [D, D])
    k^T v       = Wk^T G Wv
    ||k_e||^2   = diag(Wk^T G Wk)_e ,  ||v_e||^2 = diag(Wv^T G Wv)_e
so k and v ([N, D] each) are never materialized; the sequence-length
reduction happens once inside G (upper-triangle blocks only, G symmetric).
rnk = 1/(nk+eps) folds into the relu-copy of KV as a per-partition ACT
scale (relu commutes with positive scaling); rnv = 1/(nv+eps) is a pure
column scaling of the final output.  query is produced transposed
(query^T, [D, N]) so the final einsum consumes it as the matmul stationary
operand directly.

All big matmuls run as float32r (full PE rate; fp32 is 4x slower).  The
BIR verifier requires f32r matmul inputs to be produced as f32r, so the
feeding DRAM tensors / SBUF tiles are declared float32r (same 4-byte bits).

Sharding: data-parallel over batch — 8 batch elements -> 8 NeuronCores,
weights replicated.  No collectives.
"""

import numpy as np

import concourse.bass as bass
import concourse.mybir as mybir
import concourse.tile as tile
from concourse.bass_utils import run_bass_kernel_spmd
from concourse.masks import make_identity

F32 = mybir.dt.float32
F32R = mybir.dt.float32r
RELU = mybir.ActivationFunctionType.Relu
SQRT = mybir.ActivationFunctionType.Sqrt

B, N, D = 8, 4096, 1024
P = 128
DC = D // P          # 8 feature chunks
SLAB = 512           # token slab
NSLAB = N // SLAB    # 8
EPS = 1e-05

# G = x^T x upper-triangle piece tables: (rowblock i, colstart, width, bank, bankoff)
# Row-block i covers G[128i:128(i+1), 128i:1024]; pieces are matmul moving slices
# packed into [128, 512] f32 PSUM bank tiles.
G_PIECES_A = [  # row blocks 0..3 -> 7 banks (phase A)
    (0, 0, 512, 0, 0), (0, 512, 512, 1, 0),
    (1, 128, 512, 2, 0), (1, 640, 384, 3, 0),
    (3, 384, 128, 3, 384), (3, 512, 512, 4, 0),
    (2, 256, 512, 5, 0), (2, 768, 256, 6, 0),
]
N_BANKS_A = 7
G_PIECES_B = [  # row blocks 4..7 -> 3 banks (phase B)
    (4, 512, 512, 0, 0),
    (5, 640, 384, 1, 0), (7, 896, 128, 1, 384),
    (6, 768, 256, 2, 0),
]
N_BANKS_B = 3

last_exec_time_ns = None
last_results = None


def _split_multi_waits(nc, max_waits=1):
    """This container's walrus accepts at most ONE sync-wait per instruction
    ("Too many sync wait commands" otherwise).  Tile attaches several, so
    move the extras onto injected same-engine NoOps placed just before each
    offending instruction — engine streams dispatch in order, so a leading
    nop that blocks on the extra conditions is semantically identical."""
    ctr = 0
    for func in nc.m.functions:
        for bb in func.blocks:
            out = []
            changed = False
            for inst in bb.instructions:
                si = inst.sync_info
                waits = list(si.on_wait) if si and si.on_wait else []
                if len(waits) > max_waits:
                    for w in waits[:-max_waits]:
                        ctr += 1
                        nop = mybir.InstNoOp(
                            name=f"I-waitsplit-{ctr}",
                            engine=inst.engine,
                            sync_info=mybir.SyncInfo(on_wait=[w], on_update=[]),
                        )
                        nc.register_instruction(nop)
                        out.append(nop)
                    inst.sync_info = mybir.SyncInfo(
                        on_wait=waits[-max_waits:],
                        on_update=list(si.on_update) if si.on_update else [],
                    )
                    changed = True
                out.append(inst)
            if changed:
                bb.instructions = out
    return ctr


def _copy_r(nc, idx, out_r, in_ps):
    """PSUM -> f32r SBUF copy, alternating DVE / ACT.
    DVE path reads the source as f32r (f32r->f32r copy); ACT path reads it
    as f32 and casts on writeback (both verified on HW)."""
    if idx % 2 == 0:
        nc.vector.tensor_copy(out_r, in_ps.bitcast(F32R))
    else:
        nc.scalar.copy(out_r, in_ps.bitcast(F32))


def _load_weight(nc, pool, w_dram, name):
    """[D, D] f32r weight -> SBUF [128, DC, D] (partition = row-within-chunk),
    one DMA per row chunk so consumers can start before the full load."""
    t = pool.tile([P, DC, D], F32R, tag=name)
    wr = w_dram.rearrange("(c p) e -> p c e", p=P)
    for c in range(DC):
        nc.sync.dma_start(t[:, c, :], wr[:, c, :])
    return t


def _build_program():
    # all data DMAs here are HWDGE (nc.sync); shrink the SWDGE descriptor-ring
    # SBUF carveout from its 16 KiB/partition default to reclaim SBUF
    nc = bass.Bass(dynamic_dma_scratch_size=2048)

    x_d = nc.dram_tensor("x", [N, D], F32R, kind="ExternalInput")
    wqr_d = nc.dram_tensor("w_query_real", [D, D], F32R, kind="ExternalInput")
    wqi_d = nc.dram_tensor("w_query_imag", [D, D], F32R, kind="ExternalInput")
    wk_d = nc.dram_tensor("w_key", [D, D], F32R, kind="ExternalInput")
    wv_d = nc.dram_tensor("w_value", [D, D], F32R, kind="ExternalInput")
    y_d = nc.dram_tensor("y", [N, D], F32, kind="ExternalOutput")

    x_r = x_d.rearrange("(s t p) d -> s p t d", p=P, t=SLAB // P)  # [8, 128, 4, 1024]

    with tile.TileContext(nc) as tc:
        with (
            tc.tile_pool(name="consts", bufs=1) as consts,
            tc.tile_pool(name="gsb", bufs=1) as gsb_pool,
            tc.tile_pool(name="asb", bufs=1) as a_pool,
            tc.tile_pool(name="vecs", bufs=1) as vecs_pool,
            tc.tile_pool(name="dram", bufs=1, space="DRAM") as dram_pool,
        ):
            ident_f = consts.tile([P, P], F32)
            make_identity(nc, ident_f)
            ident = consts.tile([P, P], F32R)
            nc.sync.dma_start(ident[:], ident_f[:].bitcast(F32R))
            ones = consts.tile([P, 1], F32)
            nc.vector.memset(ones, 1.0)

            g_sb = gsb_pool.tile([P, DC, D], F32R)  # full G, filled in pieces
            rnv_b = vecs_pool.tile([P, D], F32)     # 1/(nv+eps) bcast (phase D)
            qT_dram = dram_pool.tile([D, N], F32R)  # query^T spill

            with tc.tile_pool(name="wq", bufs=1) as wq_pool:
                # the two query-projection weights (used in phase B)
                wqr_sb = _load_weight(nc, wq_pool, wqr_d, "wqr")
                wqi_sb = _load_weight(nc, wq_pool, wqi_d, "wqi")

                # ---------------- Phase A: G row-blocks 0..3 ----------------
                with (
                    tc.tile_pool(name="xa", bufs=3) as xa_pool,
                    tc.tile_pool(name="psA", bufs=1, space="PSUM") as psA,
                ):
                    ga = [psA.tile([P, 512], F32, tag=f"ga{k}", name=f"ga{k}") for k in range(N_BANKS_A)]
                    for s in range(NSLAB):
                        xa = xa_pool.tile([P, SLAB // P, D], F32R, tag="xa")
                        nc.sync.dma_start(xa[:], x_r[s])
                        for t in range(SLAB // P):
                            for (i, cs, w, bk, off) in G_PIECES_A:
                                nc.tensor.matmul(
                                    ga[bk][:, off : off + w],
                                    xa[:, t, i * P : (i + 1) * P],
                                    xa[:, t, cs : cs + w],
                                    start=(s == 0 and t == 0),
                                    stop=(s == NSLAB - 1 and t == SLAB // P - 1),
                                )
                    # flush G rows 0..3 to SBUF
                    for n_, (i, cs, w, bk, off) in enumerate(G_PIECES_A):
                        _copy_r(nc, n_, g_sb[:, i, cs : cs + w], ga[bk][:, off : off + w])

                # ------- Phase B: transpose + query^T + G rows 4..7 ---------
                with (
                    tc.tile_pool(name="xb", bufs=2) as xb_pool,
                    tc.tile_pool(name="xt", bufs=2) as xt_pool,
                    tc.tile_pool(name="qre", bufs=2) as qre_pool,
                    tc.tile_pool(name="qto", bufs=3) as qto_pool,
                    tc.tile_pool(name="psB", bufs=1, space="PSUM") as psB,
                    tc.tile_pool(name="pt", bufs=2, space="PSUM") as pt_pool,
                    tc.tile_pool(name="pq", bufs=3, space="PSUM") as pq_pool,
                ):
                    gb = [psB.tile([P, 512], F32, tag=f"gb{k}", name=f"gb{k}") for k in range(N_BANKS_B)]
                    for s in range(NSLAB):
                        n0 = s * SLAB
                        xb = xb_pool.tile([P, SLAB // P, D], F32R, tag="xb")
                        nc.sync.dma_start(xb[:], x_r[s])

                        # transpose x slab -> x^T slab [128(d), DC, SLAB]
                        xt = xt_pool.tile([P, DC, SLAB], F32R, tag="xt")
                        for dc in range(DC):
                            ptile = pt_pool.tile([P, SLAB], F32R, tag="pt", name="pt")
                            for t in range(SLAB // P):
                                nc.tensor.transpose(
                                    ptile[:, t * P : (t + 1) * P],
                                    xb[:, t, dc * P : (dc + 1) * P],
                                    ident,
                                )
                            _copy_r(nc, dc, xt[:, dc, :], ptile[:].bitcast(F32))

                        # G row-blocks 4..7 accumulation
                        for t in range(SLAB // P):
                            for (i, cs, w, bk, off) in G_PIECES_B:
                                nc.tensor.matmul(
                                    gb[bk][:, off : off + w],
                                    xb[:, t, i * P : (i + 1) * P],
                                    xb[:, t, cs : cs + w],
                                    start=(s == 0 and t == 0),
                                    stop=(s == NSLAB - 1 and t == SLAB // P - 1),
                                )

                        # query^T = relu(Wqr^T x^T) * relu(Wqi^T x^T)
                        for ec in range(DC):
                            pr = pq_pool.tile([P, SLAB], F32, tag="pq", name="pr")
                            for dc in range(DC):
                                nc.tensor.matmul(
                                    pr[:],
                                    wqr_sb[:, dc, ec * P : (ec + 1) * P],
                                    xt[:, dc, :],
                                    start=(dc == 0),
                                    stop=(dc == DC - 1),
                                )
                            pi = pq_pool.tile([P, SLAB], F32, tag="pq", name="pi")
                            for dc in range(DC):
                                nc.tensor.matmul(
                                    pi[:],
                                    wqi_sb[:, dc, ec * P : (ec + 1) * P],
                                    xt[:, dc, :],
                                    start=(dc == 0),
                                    stop=(dc == DC - 1),
                                )
                            rr = qre_pool.tile([P, SLAB], F32, tag="rr")
                            nc.scalar.activation(rr[:], pr[:], RELU)
                            ri = qre_pool.tile([P, SLAB], F32, tag="ri")
                            nc.scalar.activation(ri[:], pi[:], RELU)
                            qt = qto_pool.tile([P, SLAB], F32, tag="qt")
                            nc.vector.tensor_mul(qt[:], rr[:], ri[:])
                            nc.sync.dma_start(
                                qT_dram[ec * P : (ec + 1) * P, n0 : n0 + SLAB],
                                qt[:].bitcast(F32R),
                            )
                    # flush G rows 4..7
                    for n_, (i, cs, w, bk, off) in enumerate(G_PIECES_B):
                        _copy_r(nc, n_, g_sb[:, i, cs : cs + w], gb[bk][:, off : off + w])

            # mirror the strictly-upper blocks of G into the lower triangle
            with tc.tile_pool(name="ptC", bufs=2, space="PSUM") as ptC_pool:
                for i in range(DC):
                    for j in range(i + 1, DC):
                        ptile = ptC_pool.tile([P, P], F32R, tag="ptc", name="ptc")
                        nc.tensor.transpose(
                            ptile[:], g_sb[:, i, j * P : (j + 1) * P], ident
                        )
                        _copy_r(nc, i + j, g_sb[:, j, i * P : (i + 1) * P],
                                ptile[:].bitcast(F32))

            # ---------------- Phase C: KV attention matrix A ----------------
            with (
                tc.tile_pool(name="wkv", bufs=1) as wkv_pool,
                tc.tile_pool(name="mv", bufs=1) as mv_pool,
                tc.tile_pool(name="ctmp", bufs=2) as ctmp_pool,
                tc.tile_pool(name="cvec", bufs=1) as cvec_pool,
                tc.tile_pool(name="psC", bufs=3, space="PSUM") as psC_pool,
                tc.tile_pool(name="pnrm", bufs=1, space="PSUM") as pnrm_pool,
            ):
                a_sb = a_pool.tile([P, DC, D], F32R)  # relu'd row-scaled KV
                wk_sb = _load_weight(nc, wkv_pool, wk_d, "wk")
                wv_sb = _load_weight(nc, wkv_pool, wv_d, "wv")
                mv_sb = mv_pool.tile([P, DC, D], F32R)

                # Mv = G Wv   [D, D]
                for mc in range(DC):
                    for eh in range(2):
                        pm = psC_pool.tile([P, 512], F32, tag="psc", name="pm")
                        for dc in range(DC):
                            nc.tensor.matmul(
                                pm[:],
                                g_sb[:, dc, mc * P : (mc + 1) * P],
                                wv_sb[:, dc, eh * 512 : (eh + 1) * 512],
                                start=(dc == 0),
                                stop=(dc == DC - 1),
                            )
                        _copy_r(nc, eh, mv_sb[:, mc, eh * 512 : (eh + 1) * 512], pm[:])

                # nv^2 = colsum(Wv * Mv) ; rnv = 1/(sqrt(nv^2)+eps)
                pnv = [pnrm_pool.tile([1, 512], F32, tag=f"pnv{h}", name=f"pnv{h}") for h in range(2)]
                for mc in range(DC):
                    tmpv = ctmp_pool.tile([P, D], F32, tag="tmpv")
                    nc.vector.tensor_mul(
                        tmpv[:],
                        wv_sb[:, mc, :].bitcast(F32),
                        mv_sb[:, mc, :].bitcast(F32),
                    )
                    for eh in range(2):
                        nc.tensor.matmul(
                            pnv[eh][:],
                            ones[:, 0:1],
                            tmpv[:, eh * 512 : (eh + 1) * 512],
                            start=(mc == 0),
                            stop=(mc == DC - 1),
                        )
                rnv_row = cvec_pool.tile([1, D], F32, tag="rnv_row")
                for eh in range(2):
                    nc.scalar.activation(
                        rnv_row[:, eh * 512 : (eh + 1) * 512], pnv[eh][:], SQRT
                    )
                nc.vector.tensor_scalar_add(rnv_row[:], rnv_row[:], EPS)
                nc.vector.reciprocal(rnv_row[:], rnv_row[:])
                # broadcast [1, D] across partitions via DRAM bounce
                nv_dram = dram_pool.tile([1, D], F32)
                nc.sync.dma_start(nv_dram[:], rnv_row[:])
                nc.sync.dma_start(rnv_b[:], nv_dram[0:1, :].to_broadcast((P, D)))

                # nk^2 via Mk = G Wk (not materialized) ; rnk
                pnk = [pnrm_pool.tile([1, 512], F32, tag=f"pnk{h}", name=f"pnk{h}") for h in range(2)]
                for mc in range(DC):
                    tmpk = ctmp_pool.tile([P, D], F32, tag="tmpk")
                    for eh in range(2):
                        pk = psC_pool.tile([P, 512], F32, tag="psc", name="pk")
                        for dc in range(DC):
                            nc.tensor.matmul(
                                pk[:],
                                g_sb[:, dc, mc * P : (mc + 1) * P],
                                wk_sb[:, dc, eh * 512 : (eh + 1) * 512],
                                start=(dc == 0),
                                stop=(dc == DC - 1),
                            )
                        nc.vector.tensor_mul(
                            tmpk[:, eh * 512 : (eh + 1) * 512],
                            wk_sb[:, mc, eh * 512 : (eh + 1) * 512].bitcast(F32),
                            pk[:],
                        )
                    for eh in range(2):
                        nc.tensor.matmul(
                            pnk[eh][:],
                            ones[:, 0:1],
                            tmpk[:, eh * 512 : (eh + 1) * 512],
                            start=(mc == 0),
                            stop=(mc == DC - 1),
                        )
                rnk_row = cvec_pool.tile([1, D], F32, tag="rnk_row")
                for eh in range(2):
                    nc.scalar.activation(
                        rnk_row[:, eh * 512 : (eh + 1) * 512], pnk[eh][:], SQRT
                    )
                nc.vector.tensor_scalar_add(rnk_row[:], rnk_row[:], EPS)
                nc.vector.reciprocal(rnk_row[:], rnk_row[:])
                nk_dram = dram_pool.tile([1, D], F32)
                nc.sync.dma_start(nk_dram[:], rnk_row[:])
                # rnk in per-partition layout [128, DC]: partition p <-> e_k = ec*128+p
                rnk_part = cvec_pool.tile([P, DC], F32, tag="rnk_part")
                nc.sync.dma_start(
                    rnk_part[:], nk_dram[0:1, :].rearrange("o (c p) -> (o p) c", p=P)
                )

                # A = relu(diag(rnk) Wk^T Mv)  (rnv deferred to output columns)
                for ekc in range(DC):
                    for eh in range(2):
                        pkv = psC_pool.tile([P, 512], F32, tag="psc", name="pkv")
                        for dc in range(DC):
                            nc.tensor.matmul(
                                pkv[:],
                                wk_sb[:, dc, ekc * P : (ekc + 1) * P],
                                mv_sb[:, dc, eh * 512 : (eh + 1) * 512],
                                start=(dc == 0),
                                stop=(dc == DC - 1),
                            )
                        nc.scalar.activation(
                            a_sb[:, ekc, eh * 512 : (eh + 1) * 512],
                            pkv[:],
                            RELU,
                            scale=rnk_part[:, ekc : ekc + 1],
                        )

            # ---------------- Phase D: out = (query @ A) * rnv ----------------
            with (
                tc.tile_pool(name="qd", bufs=2) as qd_pool,
                tc.tile_pool(name="ot", bufs=3) as ot_pool,
                tc.tile_pool(name="po", bufs=4, space="PSUM") as po_pool,
            ):
                qT_r = qT_dram[:].rearrange("(c p) n -> p c n", p=P)  # [128, DC, N]
                for s in range(NSLAB):
                    n0 = s * SLAB
                    qs = qd_pool.tile([P, DC, SLAB], F32R, tag="qs")
                    nc.sync.dma_start(qs[:], qT_r[:, :, n0 : n0 + SLAB])
                    for t in range(SLAB // P):
                        ot = ot_pool.tile([P, D], F32, tag="ot")
                        for eh in range(2):
                            po = po_pool.tile([P, 512], F32, tag="po", name="po")
                            for ec in range(DC):
                                nc.tensor.matmul(
                                    po[:],
                                    qs[:, ec, t * P : (t + 1) * P],
                                    a_sb[:, ec, eh * 512 : (eh + 1) * 512],
                                    start=(ec == 0),
                                    stop=(ec == DC - 1),
                                )
                            nc.vector.tensor_mul(
                                ot[:, eh * 512 : (eh + 1) * 512],
                                po[:],
                                rnv_b[:, eh * 512 : (eh + 1) * 512],
                            )
                        nc.sync.dma_start(
                            y_d[n0 + t * P : n0 + (t + 1) * P, :], ot[:]
                        )

    _split_multi_waits(nc)
    return nc


_program_cache = None


def kernel(_trace=False, **inputs):
    global _program_cache, last_exec_time_ns, last_results
    if _program_cache is None:
        _program_cache = _build_program()
    nc = _program_cache

    x = np.ascontiguousarray(np.asarray(inputs["x"], dtype=np.float32))
    in_maps = []
    for b in range(B):
        in_maps.append(
            {
                "x": x[b],
                "w_query_real": np.asarray(inputs["w_query_real"], dtype=np.float32),
                "w_query_imag": np.asarray(inputs["w_query_imag"], dtype=np.float32),
                "w_key": np.asarray(inputs["w_key"], dtype=np.float32),
                "w_value": np.asarray(inputs["w_value"], dtype=np.float32),
            }
        )
    kwargs = {}
    if _trace:
        kwargs = dict(trace=True, tmpdir="/tmp/kernel_trace")
    res = run_bass_kernel_spmd(nc, in_maps, core_ids=list(range(B)), **kwargs)
    last_exec_time_ns = res.exec_time_ns
    last_results = res
    return np.stack([res.results[b]["y"] for b in range(B)], axis=0)
